# revision 2
# baseline (speedup 1.0000x reference)
"""GroupedVectorAttention Trainium2 kernel v3 (8-core SPMD, data-parallel).

v3 = v2 Phase B (dma_gather superrow design, unchanged) + rewritten Phase A:
the k/v/q projections run feature-major with HOST-TRANSPOSED inputs
(kT/vT/qT/xyzT), so each 512-row chunk needs only 4 weight-stationary
matmuls + 4 PE transposes (for the point-major packed-table write) instead
of v2's 12 transposes + 12 data-stationary matmuls + 12 DVE copies.
The LN rstd rides the transpose as bundle row 111 and is applied to the
bkey fields point-major afterwards.
"""

import numpy as np
from contextlib import ExitStack

import concourse.bass as bass
import concourse.bacc as bacc
import concourse.tile as tile
from concourse import mybir
from concourse.bass_utils import run_bass_kernel_spmd
from concourse.masks import make_identity

# ---------------------------------------------------------------------------
# Workaround: this walrus build rejects PE Matmult instructions carrying more
# than one semaphore wait ("Too many sync wait commands").  Split extra waits
# onto NoOp carrier instructions on the same engine queue, placed immediately
# before the matmul, right after Tile's wait-assignment pass.
_orig_postorder = tile.postorder_instruction_blocks
_nop_ctr = [0]


def _split_pe_waits(ordered, *args, **kwargs):
    for bb, insts in list(ordered.items()):
        out = []
        for inst in insts:
            si = getattr(inst, "sync_info", None)
            if (si is not None and si.on_wait and len(si.on_wait) > 1
                    and inst.engine != mybir.EngineType.Pool):
                waits = list(si.on_wait)
                for w in waits[:-1]:
                    _nop_ctr[0] += 1
                    nop = mybir.InstNoOp(name=f"nopw-{_nop_ctr[0]}")
                    nop.engine = inst.engine
                    nop.sync_info = mybir.SyncInfo(on_wait=[w], on_update=[])
                    out.append(nop)
                inst.sync_info = mybir.SyncInfo(
                    on_wait=[waits[-1]], on_update=list(si.on_update)
                )
            out.append(inst)
        ordered[bb] = out
    return _orig_postorder(ordered, *args, **kwargs)


tile.postorder_instruction_blocks = _split_pe_waits

from concourse.vector_clock import ScopedClock as _ScopedClock


def _patched_drain_and_barrier(self, tick_clock, wait_clock):
    probe = self.nc.sync.nop(nofuse=True)
    wait_clock.add_sem_waits(
        probe.ins, _ScopedClock({None: tick_clock.global_clock})
    )
    si = probe.ins.sync_info
    if si is not None and si.on_wait and len(si.on_wait) > 1:
        waits = list(si.on_wait)
        probe.ins.sync_info = mybir.SyncInfo(
            on_wait=waits[:1], on_update=list(si.on_update)
        )
        for w in waits[1:]:
            n2 = self.nc.sync.nop(nofuse=True)
            n2.ins.sync_info = mybir.SyncInfo(on_wait=[w], on_update=[])
    self.nc.sync.drain()
    self.nc.all_engine_barrier()
    popped = self.nc._tile_sem_poison_stack.pop()
    assert popped is self._sem_poison
    self.nc.clear_and_free_semaphores(list(self.sems.allocated().values()))
    self.nc.all_engine_barrier()


tile.TileContext._drain_and_barrier = _patched_drain_and_barrier

P = 128
C = 96
G = 12
S = 16
CG = C // G  # 8
EPS = 1e-5
SRE = 256        # fp16 elems per 512B superrow: [row0 112|pad16|row1 112|pad16]
F16 = mybir.dt.float16
F32 = mybir.dt.float32
I16 = mybir.dt.int16
AX = mybir.AxisListType.X
ALU = mybir.AluOpType
ACTF = mybir.ActivationFunctionType


def _build(NR, NT, debug=False):
    """Per-core Bacc kernel. NR = padded rows per core (mult of 512),
    NT = padded table rows (mult of 512)."""
    assert NR % 512 == 0 and NT % 1024 == 0
    NT2 = NT // 2
    nc = bacc.Bacc(dynamic_dma_scratch_size=65536, num_swdge_queues=2)

    kT_d = nc.declare_dram_parameter("kT", [C, NT], F16, isOutput=False)
    vT_d = nc.declare_dram_parameter("vT", [C, NT], F16, isOutput=False)
    xyzT_d = nc.declare_dram_parameter("xyzT", [3, NT], F16, isOutput=False)
    qT_d = nc.declare_dram_parameter("qT", [C, NR], F16, isOutput=False)
    xyzsT_d = nc.declare_dram_parameter("xyzsT", [3, NR], F16, isOutput=False)
    idx16_d = nc.declare_dram_parameter("idx16", [NR, P], I16, isOutput=False)
    par_d = nc.declare_dram_parameter("par", [NR, S], F16, isOutput=False)
    Wq_d = nc.declare_dram_parameter("Wqc", [C, C], F16, isOutput=False)
    Wk_d = nc.declare_dram_parameter("Wkc", [C, C], F16, isOutput=False)
    Wv_d = nc.declare_dram_parameter("Wv", [C, C], F16, isOutput=False)
    Ww1_d = nc.declare_dram_parameter("Ww1c", [C, G], F16, isOutput=False)
    Wp1blk_d = nc.declare_dram_parameter("Wp1blk", [3 * S, S * C], F16,
                                         isOutput=False)
    MqB_d = nc.declare_dram_parameter("MqB", [3 * S, 3 * S], F16, isOutput=False)
    Wp2_d = nc.declare_dram_parameter("Wp2", [C, C], F16, isOutput=False)
    Wp2w1_d = nc.declare_dram_parameter("Wp2w1", [C, G], F16, isOutput=False)
    Ww2B_d = nc.declare_dram_parameter("Ww2B", [8 * G, 8 * G], F16, isOutput=False)
    out = nc.declare_dram_parameter("out", [NR, C], F32, isOutput=True)

    packed = nc.dram_tensor("packed", [NT2, SRE], F16)
    qpack = nc.dram_tensor("qpack", [NR, 16], F16)

    with ExitStack() as ctx:
        tc = ctx.enter_context(tile.TileContext(nc))
        consts = ctx.enter_context(tc.tile_pool(name="consts", bufs=1))
        # SBUF pools
        sb_in = ctx.enter_context(tc.tile_pool(name="sb_in", bufs=3))
        sb_t = ctx.enter_context(tc.tile_pool(name="sb_t", bufs=3))
        sb_st = ctx.enter_context(tc.tile_pool(name="sb_st", bufs=3))
        sb_sm = ctx.enter_context(tc.tile_pool(name="sb_sm", bufs=4))
        sb_g = ctx.enter_context(tc.tile_pool(name="sb_g", bufs=5))
        sb_b = ctx.enter_context(tc.tile_pool(name="sb_b", bufs=3))

        ident = consts.tile([P, P], F16)
        make_identity(nc, ident[:])
        epst = consts.tile([P, 1], F32)
        nc.vector.memset(epst[:], EPS)
        ones96 = consts.tile([C, 1], F16)
        nc.vector.memset(ones96[:], 1.0)
        ones12 = consts.tile([1, G], F16)
        nc.vector.memset(ones12[:], 1.0)

        def load_const(name, dram, shape):
            t = consts.tile(shape, F16, tag=name)
            nc.sync.dma_start(out=t[:], in_=dram[:])
            return t

        wq_sb = load_const("wq", Wq_d, [C, C])
        wk_sb = load_const("wk", Wk_d, [C, C])
        wv_sb = load_const("wv", Wv_d, [C, C])
        ww1_sb = load_const("ww1", Ww1_d, [C, G])
        wp1b_sb = load_const("wp1b", Wp1blk_d, [3 * S, S * C])
        mqb_sb = load_const("mqb", MqB_d, [3 * S, 3 * S])
        wp2_sb = load_const("wp2", Wp2_d, [C, C])
        wp2w1_sb = load_const("wp2w1", Wp2w1_d, [C, C // CG])
        ww2b_sb = load_const("ww2b", Ww2B_d, [8 * G, 8 * G])

        # ---------------- Phase A: packed superrow table (feature-major) ------
        # Per 512-point chunk: y=Wkc^T.kT  (LN via ssq->rstd), b0=Ww1c^T.relu(y),
        # vv=Wv^T.vT.  bundle [112,512] = [vv 96 | b0 12 | xyz 3 | rstd 1];
        # 4 PE transposes (strided cols j::4) -> point-major stg; bkey scaled
        # by rstd post-transpose; DMA to packed.
        actx = ExitStack()
        pp_y = actx.enter_context(tc.tile_pool(name="pa_y", bufs=2, space="PSUM"))
        pp_v = actx.enter_context(tc.tile_pool(name="pa_v", bufs=2, space="PSUM"))
        pp_tp = actx.enter_context(tc.tile_pool(name="pa_tp", bufs=2, space="PSUM"))
        pp_w = actx.enter_context(tc.tile_pool(name="pa_w", bufs=2, space="PSUM"))

        def chunk_fm(src_dram, w_sb, c0, with_val):
            """Returns bundle [112, 512] f16 (val|b0|xyz|rstd rows) or
            [16, 512] for the q-side (a0|xyz|rstd)."""
            xc = sb_in.tile([C, 512], F16, tag="xc")
            nc.sync.dma_start(out=xc[:], in_=src_dram[:, c0:c0 + 512])
            y = pp_y.tile([C, 512], F32, tag="y")
            nc.tensor.matmul(out=y[:], lhsT=w_sb[:], rhs=xc[:],
                             start=True, stop=True)
            rk = sb_t.tile([C, 512], F16, tag="rk")
            nc.vector.tensor_scalar_max(out=rk[:], in0=y[:], scalar1=0.0)
            sq = sb_sm.tile([C, 512], F16, tag="sq")
            nc.scalar.activation(out=sq[:], in_=y[:], func=ACTF.Square)
            wb = pp_w.tile([33, 512], F32, tag="wb")
            nc.tensor.matmul(out=wb[32:33, :], lhsT=ones96[:], rhs=sq[:],
                             start=True, stop=True)
            nc.tensor.matmul(out=wb[0:G, :], lhsT=ww1_sb[:], rhs=rk[:],
                             start=True, stop=True)
            ssqs = sb_sm.tile([1, 512], F16, tag="ssqs")
            nc.vector.tensor_copy(out=ssqs[:], in_=wb[32:33, :])
            return wb, ssqs

        for b in range(NT // 512):
            c0 = b * 512
            sr0 = c0 // 2
            bundle = sb_st.tile([112, 512], F16, tag="bun")
            nc.sync.dma_start(out=bundle[108:111, :],
                              in_=xyzT_d[:, c0:c0 + 512])
            wb, ssqs = chunk_fm(kT_d, wk_sb, c0, True)
            nc.vector.tensor_copy(out=bundle[96:108, :], in_=wb[0:G, :])
            nc.sync.dma_start(out=bundle[111:112, :], in_=ssqs[:])
            vc = sb_in.tile([C, 512], F16, tag="vc")
            nc.sync.dma_start(out=vc[:], in_=vT_d[:, c0:c0 + 512])
            vv = pp_v.tile([C, 512], F32, tag="v")
            nc.tensor.matmul(out=vv[:], lhsT=wv_sb[:], rhs=vc[:],
                             start=True, stop=True)
            nc.scalar.copy(out=bundle[0:96, :], in_=vv[:])

            tp = pp_tp.tile([P, 4, 112], F16, tag="tp")
            bview = bundle[:].rearrange("f (p j) -> f j p", j=4)
            for j in range(4):
                nc.tensor.transpose(out=tp[:, j, :], in_=bview[:, j, :],
                                    identity=ident[0:112, 0:112])
            stg = sb_st.tile([P, 2, SRE], F16, tag="stg")
            stg4 = stg[:].rearrange("p a (o x) -> p (a o) x", o=2)  # [P,4,128]
            nc.vector.tensor_copy(out=stg4[:, :, 0:112], in_=tp[:])
            nc.sync.dma_start(
                out=packed[sr0:sr0 + 256, :].rearrange("(p a) e -> p a e", a=2),
                in_=stg[:])

        # ---------------- Phase A2: qpack [aq 12 | xyz 3 | rstd] --------------
        for b in range(NR // 512):
            c0 = b * 512
            bq = sb_st.tile([16, 512], F16, tag="bq")
            nc.sync.dma_start(out=bq[12:15, :], in_=xyzsT_d[:, c0:c0 + 512])
            wb, ssqs = chunk_fm(qT_d, wq_sb, c0, False)
            nc.vector.tensor_copy(out=bq[0:G, :], in_=wb[0:G, :])
            nc.sync.dma_start(out=bq[15:16, :], in_=ssqs[:])
            tpq_full = pp_tp.tile([P, 4, 112], F16, tag="tp")
            tpq = tpq_full[:, :, 0:16]
            bqv = bq[:].rearrange("f (p j) -> f j p", j=4)
            for j in range(4):
                nc.tensor.transpose(out=tpq[:, j, :], in_=bqv[:, j, :],
                                    identity=ident[0:16, 0:16])
            qstg = sb_st.tile([P, 4, 16], F16, tag="qstg")
            nc.vector.tensor_copy(out=qstg[:], in_=tpq[:])
            nc.sync.dma_start(
                out=qpack[c0:c0 + 512, :].rearrange("(p a) e -> p a e", a=4),
                in_=qstg[:])

        actx.close()
        # PSUM pools (8 banks: 2+2+2+2)
        pp_y = ctx.enter_context(tc.tile_pool(name="pp_y", bufs=2, space="PSUM"))
        pp_v = ctx.enter_context(tc.tile_pool(name="pp_v", bufs=2, space="PSUM"))
        pp_tp = ctx.enter_context(tc.tile_pool(name="pp_tp", bufs=2, space="PSUM"))
        pp_w = ctx.enter_context(tc.tile_pool(name="pp_w", bufs=2, space="PSUM"))

        # ---------------- Phase B: per 128-point tile --------------------------
        for t in range(NR // P):
            r0 = t * P
            qp = sb_sm.tile([P, 16], F16, tag="qp")
            nc.sync.dma_start(out=qp[:], in_=qpack[r0:r0 + P, :])
            pr = sb_sm.tile([P, S], F16, tag="pr")
            nc.sync.dma_start(out=pr[:], in_=par_d[r0:r0 + P, :])
            ixt = sb_sm.tile([P, P], I16, tag="ixt")
            nc.sync.dma_start(out=ixt[:], in_=idx16_d[r0:r0 + P, :])
            Gt = sb_g.tile([P, S, SRE], F16, tag="G")
            nc.gpsimd.dma_gather(
                out_ap=Gt[:], in_ap=packed[:, :], idxs_ap=ixt[:],
                num_idxs=P * S, num_idxs_reg=P * S, elem_size=SRE,
                single_packet=False, queue_num=t % 2)
            Gpair = Gt[:].rearrange("p s (o x) -> p s o x", o=2)

            # parity-select the 16 small fields: [b0 12 | xyz 3 | ssq 1]
            prb16 = pr[:].rearrange("p (s o) -> p s o", o=1).broadcast_to([P, S, 16])
            d15 = sb_sm.tile([P, S, 16], F16, tag="d15")
            nc.vector.tensor_tensor(out=d15[:], in0=Gpair[:, :, 1, 96:112],
                                    in1=Gpair[:, :, 0, 96:112], op=ALU.subtract)
            nc.vector.tensor_tensor(out=d15[:], in0=d15[:], in1=prb16,
                                    op=ALU.mult)
            sel = sb_sm.tile([P, S, 16], F16, tag="sel")
            nc.vector.tensor_tensor(out=sel[:], in0=Gpair[:, :, 0, 96:112],
                                    in1=d15[:], op=ALU.add)
            # k-side LN scale: bks = b0 * rsqrt(ssq/C + eps)
            sdk = sb_sm.tile([P, S], F32, tag="sdk")
            nc.scalar.activation(out=sdk[:],
                                 in_=sel[:, :, 15:16].rearrange("p s o -> p (s o)"),
                                 func=ACTF.Sqrt, scale=1.0 / C, bias=epst[:])
            rkk = sb_sm.tile([P, S], F32, tag="rkk")
            nc.vector.reciprocal(out=rkk[:], in_=sdk[:])
            bks = sb_sm.tile([P, S, G], F16, tag="bks")
            nc.vector.tensor_tensor(
                out=bks[:], in0=sel[:, :, 0:12],
                in1=rkk[:].rearrange("p (s o) -> p s o", o=1)
                    .broadcast_to([P, S, G]),
                op=ALU.mult)
            # q-side LN scale: aqs = aq0 * rsqrt(ssq_q/C + eps)
            sdq = sb_sm.tile([P, 1], F32, tag="sdq")
            nc.scalar.activation(out=sdq[:], in_=qp[:, 15:16],
                                 func=ACTF.Sqrt, scale=1.0 / C, bias=epst[:])
            rkq = sb_sm.tile([P, 1], F32, tag="rkq")
            nc.vector.reciprocal(out=rkq[:], in_=sdq[:])
            aqs = sb_sm.tile([P, G], F16, tag="aqs")
            nc.vector.tensor_tensor(out=aqs[:], in0=qp[:, 0:12],
                                    in1=rkq[:].broadcast_to([P, G]),
                                    op=ALU.mult)

            # pos and its transpose
            ps = sb_sm.tile([P, S, 3], F16, tag="ps")
            nc.vector.tensor_tensor(
                out=ps[:], in0=sel[:, :, 12:15],
                in1=qp[:, 12:15].rearrange("p (o c) -> p o c", o=1)
                    .broadcast_to([P, S, 3]),
                op=ALU.subtract)
            posTp = pp_tp.tile([3 * S, P], F16, tag="tp")
            nc.tensor.transpose(out=posTp[:], in_=ps[:].rearrange("p s c -> p (s c)"),
                                identity=ident[:])
            posT = sb_t.tile([3 * S, P], F16, tag="posT")
            nc.vector.tensor_copy(out=posT[:], in_=posTp[:])

            # rstd via 3x3 Gram quadratic form: ssq = sum_c (pos@Wp1c)^2
            qf = pp_w.tile([P, 3 * S], F32, tag="w")
            nc.tensor.matmul(out=qf[:], lhsT=posT[:], rhs=mqb_sb[:],
                             start=True, stop=True)
            s2 = sb_sm.tile([P, S, 3], F16, tag="s2")
            nc.vector.tensor_tensor(
                out=s2[:], in0=ps[:],
                in1=qf[:].rearrange("p (s c) -> p s c", c=3), op=ALU.mult)
            ssqp = sb_sm.tile([P, S], F32, tag="ssqp")
            nc.vector.tensor_reduce(out=ssqp[:], in_=s2[:], axis=AX, op=ALU.add)
            sdp = sb_sm.tile([P, S], F32, tag="sdp")
            nc.scalar.activation(out=sdp[:], in_=ssqp[:], func=ACTF.Sqrt,
                                 scale=1.0 / C, bias=epst[:])
            rstd = sb_sm.tile([P, S], F32, tag="rstd")
            nc.vector.reciprocal(out=rstd[:], in_=sdp[:])
            rstdb = rstd[:].rearrange("p (s o) -> p s o", o=1)

            # pu matmuls (block-diag Wp1 stationaries) + relu -> pLT
            pLT = sb_b.tile([C, S, P], F16, tag="pLT")
            for g4 in range(4):
                puP = pp_y.tile([C, 4, P], F32, tag="y")
                for j in range(4):
                    s = g4 * 4 + j
                    nc.tensor.matmul(out=puP[:, j, :],
                                     lhsT=wp1b_sb[:, s * C:(s + 1) * C],
                                     rhs=posT[:], start=True, stop=True)
                nc.scalar.activation(out=pLT[:, g4 * 4:(g4 + 1) * 4, :],
                                     in_=puP[:], func=ACTF.Relu)

            # pwa = relu(pu) @ (Wp2@Ww1c)  [the 12-dim weight-branch pos term]
            pwaP = pp_w.tile([P, S, G], F32, tag="w")
            for s in range(S):
                nc.tensor.matmul(out=pwaP[:, s, :], lhsT=pLT[:, s, :],
                                 rhs=wp2w1_sb[:], start=True, stop=True)

            # weight branch: yt = (bkey_g - aq) + rstd*pwa ; LN_G ; relu
            yt = sb_sm.tile([P, S, G], F16, tag="yt")
            nc.vector.tensor_tensor(
                out=yt[:], in0=bks[:],
                in1=aqs[:].rearrange("p (o c) -> p o c", o=1)
                    .broadcast_to([P, S, G]),
                op=ALU.subtract)
            tyr = sb_sm.tile([P, S, G], F16, tag="tyr")
            nc.vector.tensor_tensor(out=tyr[:], in0=pwaP[:],
                                    in1=rstdb.broadcast_to([P, S, G]), op=ALU.mult)
            nc.vector.tensor_tensor(out=yt[:], in0=yt[:], in1=tyr[:], op=ALU.add)
            sqg = sb_sm.tile([P, S, G], F16, tag="sqg")
            nc.scalar.activation(out=sqg[:], in_=yt[:], func=ACTF.Square)
            ssqg = sb_sm.tile([P, S], F32, tag="ssqg")
            nc.vector.tensor_reduce(out=ssqg[:], in_=sqg[:], axis=AX, op=ALU.add)
            sdg = sb_sm.tile([P, S], F32, tag="sdg")
            nc.scalar.activation(out=sdg[:], in_=ssqg[:], func=ACTF.Sqrt,
                                 scale=1.0 / G, bias=epst[:])
            rsg = sb_sm.tile([P, S], F32, tag="rsg")
            nc.vector.reciprocal(out=rsg[:], in_=sdg[:])
            yh = sb_sm.tile([P, S, G], F16, tag="yh")
            nc.vector.tensor_tensor(
                out=yh[:], in0=yt[:],
                in1=rsg[:].rearrange("p (s o) -> p s o", o=1)
                    .broadcast_to([P, S, G]),
                op=ALU.mult)
            nc.vector.tensor_scalar_max(out=yh[:], in0=yh[:], scalar1=0.0)

            # z = yh @ kron(I8, Ww2) -> e = exp(z)
            yflat = yh[:].rearrange("p s g -> p (s g)")
            yT = sb_t.tile([C, 2, P], F16, tag="yT")
            for h in range(2):
                yhTp = pp_tp.tile([C, P], F16, tag="tp")
                nc.tensor.transpose(out=yhTp[:], in_=yflat[:, h * C:(h + 1) * C],
                                    identity=ident[:])
                nc.scalar.copy(out=yT[:, h, :], in_=yhTp[:])
            zP = pp_w.tile([P, 2, C], F32, tag="w")
            for h in range(2):
                nc.tensor.matmul(out=zP[:, h, :], lhsT=yT[:, h, :],
                                 rhs=ww2b_sb[:], start=True, stop=True)
            e = sb_sm.tile([P, S, G], F16, tag="e")
            nc.scalar.activation(out=e[:].rearrange("p s g -> p (s g)"),
                                 in_=zP[:].rearrange("p a c -> p (a c)"),
                                 func=ACTF.Exp)
            es = sb_sm.tile([P, G], F32, tag="es")
            nc.vector.tensor_reduce(out=es[:], in_=e[:].rearrange("p s g -> p g s"),
                                    axis=AX, op=ALU.add)
            rq = sb_sm.tile([P, G], F32, tag="rq")
            nc.vector.reciprocal(out=rq[:], in_=es[:])

            # parity-masked weights for the value field + u for the peb field
            ep = sb_sm.tile([P, S, 2, G], F16, tag="ep")
            nc.vector.tensor_tensor(
                out=ep[:, :, 1, :], in0=e[:],
                in1=pr[:].rearrange("p (s o) -> p s o", o=1).broadcast_to([P, S, G]),
                op=ALU.mult)
            nc.vector.tensor_tensor(out=ep[:, :, 0, :], in0=e[:],
                                    in1=ep[:, :, 1, :], op=ALU.subtract)
            u = sb_sm.tile([P, S, G], F16, tag="u")
            nc.vector.tensor_tensor(out=u[:], in0=e[:],
                                    in1=rstdb.broadcast_to([P, S, G]), op=ALU.mult)

            # weighted sums: macc = sum of e'*val(parity) and u*pebraw
            macc = sb_b.tile([P, S, C], F16, tag="macc")
            nc.vector.tensor_tensor(
                out=macc[:].rearrange("p s (g o) -> p s g o", o=CG),
                in0=Gpair[:, :, 0, 0:96].rearrange("p s (g o) -> p s g o", o=CG),
                in1=ep[:, :, 0, :].rearrange("p s (g o) -> p s g o", o=1)
                    .broadcast_to([P, S, G, CG]),
                op=ALU.mult)
            m1b = sb_b.tile([P, S, C], F16, tag="m1b")
            nc.vector.tensor_tensor(
                out=m1b[:].rearrange("p s (g o) -> p s g o", o=CG),
                in0=Gpair[:, :, 1, 0:96].rearrange("p s (g o) -> p s g o", o=CG),
                in1=ep[:, :, 1, :].rearrange("p s (g o) -> p s g o", o=1)
                    .broadcast_to([P, S, G, CG]),
                op=ALU.mult)
            nc.vector.tensor_tensor(out=macc[:], in0=macc[:], in1=m1b[:],
                                    op=ALU.add)

            m2 = sb_b.tile([P, S, C], F16, tag="m2")
            for g4 in range(4):
                pebP = pp_v.tile([P, 4, C], F32, tag="v")
                for j in range(4):
                    s = g4 * 4 + j
                    nc.tensor.matmul(out=pebP[:, j, :], lhsT=pLT[:, s, :],
                                     rhs=wp2_sb[:], start=True, stop=True)
                nc.vector.tensor_tensor(
                    out=m2[:, g4 * 4:(g4 + 1) * 4, :]
                        .rearrange("p s (g o) -> p s g o", o=CG),
                    in0=pebP[:].rearrange("p s (g o) -> p s g o", o=CG),
                    in1=u[:, g4 * 4:(g4 + 1) * 4, :]
                        .rearrange("p s (g o) -> p s g o", o=1)
                        .broadcast_to([P, 4, G, CG]),
                    op=ALU.mult)
            nc.vector.tensor_tensor(out=macc[:], in0=macc[:], in1=m2[:],
                                    op=ALU.add)

            for hw_ in (8, 4, 2, 1):
                nc.vector.tensor_tensor(out=macc[:, 0:hw_, :],
                                        in0=macc[:, 0:hw_, :],
                                        in1=macc[:, hw_:2 * hw_, :], op=ALU.add)
            fo = sb_sm.tile([P, C], F32, tag="fo")
            nc.vector.tensor_tensor(
                out=fo[:].rearrange("p (g o) -> p g o", o=CG),
                in0=macc[:, 0, :].rearrange("p (g o) -> p g o", o=CG),
                in1=rq[:].rearrange("p (g o) -> p g o", o=1).broadcast_to([P, G, CG]),
                op=ALU.mult)
            nc.sync.dma_start(out=out[r0:r0 + P, :], in_=fo[:])

    nc.finalize()
    return nc


def _center(W):
    """Remove the mean over the output axis (last)."""
    W = np.asarray(W, np.float64)
    return (W - W.mean(axis=-1, keepdims=True)).astype(np.float32)


def _prep_host(q, k, v, xyz, reference_index,
               Wq, bq, gq, betaq, Wk, bk, gk, betak, Wv, bv,
               Wp1, bp1, gp, betap, Wp2, bp2, Ww1, bw1, gw, betaw, Ww2, bw2,
               n_cores):
    for name, arr, val in [
        ("bq", bq, 0), ("gq", gq, 1), ("betaq", betaq, 0),
        ("bk", bk, 0), ("gk", gk, 1), ("betak", betak, 0),
        ("bv", bv, 0), ("bp1", bp1, 0), ("gp", gp, 1), ("betap", betap, 0),
        ("bp2", bp2, 0), ("bw1", bw1, 0), ("gw", gw, 1), ("betaw", betaw, 0),
        ("bw2", bw2, 0),
    ]:
        if not np.allclose(np.asarray(arr), val, atol=1e-6):
            raise NotImplementedError(f"non-trivial {name} not supported")

    N = q.shape[0]
    NR = ((N // n_cores) + 511) // 512 * 512
    NT = (N + 1023) // 1024 * 1024

    def padT(a, rows, dtype=np.float16):
        out = np.zeros((rows, a.shape[1]), dtype=dtype)
        out[:a.shape[0]] = np.asarray(a)
        return out

    kT = np.ascontiguousarray(padT(k, NT).T)      # [C, NT] f16
    vT = np.ascontiguousarray(padT(v, NT).T)
    xyzT = np.ascontiguousarray(padT(xyz, NT).T)  # [3, NT]

    Wq32 = _center(Wq)
    Wk32 = _center(Wk)
    Ww1c = _center(Ww1)
    Wp1c = _center(Wp1)                       # [3, C]
    Wp1c16 = Wp1c.astype(np.float16)
    M3 = (Wp1c16.astype(np.float32) @ Wp1c16.astype(np.float32).T)  # [3,3]
    MqB = np.kron(np.eye(S, dtype=np.float32), M3)                  # [48,48]
    Wp1blk = np.zeros((3 * S, S * C), np.float32)
    for s in range(S):
        Wp1blk[3 * s:3 * s + 3, s * C:(s + 1) * C] = Wp1c
    weights = {
        "Wqc": Wq32.astype(np.float16),
        "Wkc": Wk32.astype(np.float16),
        "Wv": np.asarray(Wv, np.float32).astype(np.float16),
        "Ww1c": Ww1c.astype(np.float16),
        "Wp1blk": Wp1blk.astype(np.float16),
        "MqB": MqB.astype(np.float16),
        "Wp2": np.asarray(Wp2, np.float32).astype(np.float16),
        "Wp2w1": (np.asarray(Wp2, np.float32) @ Ww1c).astype(np.float16),
        "Ww2B": np.kron(np.eye(8, dtype=np.float32),
                        np.asarray(Ww2, np.float32)).astype(np.float16),
    }

    per_core = N // n_cores
    assert per_core * n_cores == N
    ref = np.asarray(reference_index, np.int64)
    in_maps = []
    for i in range(n_cores):
        lo, hi = i * per_core, (i + 1) * per_core
        rsl = ref[lo:hi]                       # [per_core, S]
        nt_tiles = NR // P
        # idx16[t*128+p-like rows, 128]: per 128-row tile, int16 half-indices
        # ordered so gather pair j = s*128+p -> idxs[(j%16) within 16-part
        # block replicated 8x, j//16].
        idx16 = np.zeros((NR, P), np.int16)
        par = np.zeros((NR, S), np.float16)
        half = np.zeros((NR, S), np.int16)
        half[:per_core] = (rsl >> 1).astype(np.int16)
        par[:per_core] = (rsl & 1).astype(np.float16)
        for t in range(nt_tiles):
            blk = half[t * P:(t + 1) * P]          # [128, S]
            lin = blk.T.reshape(-1)                # j = s*128+p
            i16 = lin.reshape(P, 16).T             # [16, 128]
            idx16[t * P:(t + 1) * P] = np.tile(i16, (8, 1))
        m = {
            "kT": kT, "vT": vT, "xyzT": xyzT,
            "qT": np.ascontiguousarray(padT(q[lo:hi], NR).T),
            "xyzsT": np.ascontiguousarray(padT(xyz[lo:hi], NR).T),
            "idx16": idx16,
            "par": par,
        }
        m.update(weights)
        in_maps.append(m)
    return in_maps, NR, NT, per_core


_CACHE = {}


def kernel(**inputs):
    n_cores = 8
    in_maps, NR, NT, per_core = _prep_host(n_cores=n_cores, **inputs)
    key = (NR, NT)
    if key not in _CACHE:
        _CACHE[key] = _build(NR, NT)
    nc = _CACHE[key]
    res = run_bass_kernel_spmd(nc, in_maps, list(range(n_cores)))
    outs = [res.results[i]["out"][:per_core] for i in range(n_cores)]
    return np.ascontiguousarray(np.concatenate(outs, axis=0), dtype=np.float32)


# revision 3
# speedup vs baseline: 1.3141x; 1.3141x over previous
"""GroupedVectorAttention Trainium2 kernel v3 (8-core SPMD, data-parallel).

v3 = v2 Phase B (dma_gather superrow design, unchanged) + rewritten Phase A:
the k/v/q projections run feature-major with HOST-TRANSPOSED inputs
(kT/vT/qT/xyzT), so each 512-row chunk needs only 4 weight-stationary
matmuls + 4 PE transposes (for the point-major packed-table write) instead
of v2's 12 transposes + 12 data-stationary matmuls + 12 DVE copies.
The LN rstd rides the transpose as bundle row 111 and is applied to the
bkey fields point-major afterwards.
"""

import numpy as np
from contextlib import ExitStack

import concourse.bass as bass
import concourse.bacc as bacc
import concourse.tile as tile
from concourse import mybir
from concourse.bass_utils import run_bass_kernel_spmd
from concourse.masks import make_identity

# ---------------------------------------------------------------------------
# Workaround: this walrus build rejects PE Matmult instructions carrying more
# than one semaphore wait ("Too many sync wait commands").  Split extra waits
# onto NoOp carrier instructions on the same engine queue, placed immediately
# before the matmul, right after Tile's wait-assignment pass.
_orig_postorder = tile.postorder_instruction_blocks
_nop_ctr = [0]


def _split_pe_waits(ordered, *args, **kwargs):
    for bb, insts in list(ordered.items()):
        out = []
        for inst in insts:
            si = getattr(inst, "sync_info", None)
            if (si is not None and si.on_wait and len(si.on_wait) > 1
                    and inst.engine != mybir.EngineType.Pool):
                waits = list(si.on_wait)
                for w in waits[:-1]:
                    _nop_ctr[0] += 1
                    nop = mybir.InstNoOp(name=f"nopw-{_nop_ctr[0]}")
                    nop.engine = inst.engine
                    nop.sync_info = mybir.SyncInfo(on_wait=[w], on_update=[])
                    out.append(nop)
                inst.sync_info = mybir.SyncInfo(
                    on_wait=[waits[-1]], on_update=list(si.on_update)
                )
            out.append(inst)
        ordered[bb] = out
    return _orig_postorder(ordered, *args, **kwargs)


tile.postorder_instruction_blocks = _split_pe_waits

from concourse.vector_clock import ScopedClock as _ScopedClock


def _patched_drain_and_barrier(self, tick_clock, wait_clock):
    probe = self.nc.sync.nop(nofuse=True)
    wait_clock.add_sem_waits(
        probe.ins, _ScopedClock({None: tick_clock.global_clock})
    )
    si = probe.ins.sync_info
    if si is not None and si.on_wait and len(si.on_wait) > 1:
        waits = list(si.on_wait)
        probe.ins.sync_info = mybir.SyncInfo(
            on_wait=waits[:1], on_update=list(si.on_update)
        )
        for w in waits[1:]:
            n2 = self.nc.sync.nop(nofuse=True)
            n2.ins.sync_info = mybir.SyncInfo(on_wait=[w], on_update=[])
    self.nc.sync.drain()
    self.nc.all_engine_barrier()
    popped = self.nc._tile_sem_poison_stack.pop()
    assert popped is self._sem_poison
    self.nc.clear_and_free_semaphores(list(self.sems.allocated().values()))
    self.nc.all_engine_barrier()


tile.TileContext._drain_and_barrier = _patched_drain_and_barrier

P = 128
C = 96
G = 12
S = 16
CG = C // G  # 8
EPS = 1e-5
SRE = 256        # fp16 elems per 512B superrow: [row0 112|pad16|row1 112|pad16]
F16 = mybir.dt.float16
F32 = mybir.dt.float32
I16 = mybir.dt.int16
AX = mybir.AxisListType.X
ALU = mybir.AluOpType
ACTF = mybir.ActivationFunctionType


def _build(NR, NT, debug=False):
    """Per-core Bacc kernel. NR = padded rows per core (mult of 512),
    NT = padded table rows (mult of 512)."""
    assert NR % 512 == 0 and NT % 1024 == 0
    NT2 = NT // 2
    nc = bacc.Bacc(dynamic_dma_scratch_size=65536, num_swdge_queues=2)

    kT_d = nc.declare_dram_parameter("kT", [C, NT], F16, isOutput=False)
    vT_d = nc.declare_dram_parameter("vT", [C, NT], F16, isOutput=False)
    xyzT_d = nc.declare_dram_parameter("xyzT", [3, NT], F16, isOutput=False)
    qT_d = nc.declare_dram_parameter("qT", [C, NR], F16, isOutput=False)
    xyzsT_d = nc.declare_dram_parameter("xyzsT", [3, NR], F16, isOutput=False)
    idx16_d = nc.declare_dram_parameter("idx16", [NR, P], I16, isOutput=False)
    par_d = nc.declare_dram_parameter("par", [NR, S], F16, isOutput=False)
    Wq_d = nc.declare_dram_parameter("Wqc", [C, C], F16, isOutput=False)
    Wk_d = nc.declare_dram_parameter("Wkc", [C, C], F16, isOutput=False)
    Wv_d = nc.declare_dram_parameter("Wv", [C, C], F16, isOutput=False)
    Ww1_d = nc.declare_dram_parameter("Ww1c", [C, G], F16, isOutput=False)
    Wp1blk_d = nc.declare_dram_parameter("Wp1blk", [3 * S, S * C], F16,
                                         isOutput=False)
    MqB_d = nc.declare_dram_parameter("MqB", [3 * S, 3 * S], F16, isOutput=False)
    Wp2_d = nc.declare_dram_parameter("Wp2", [C, C], F16, isOutput=False)
    Wp2w1_d = nc.declare_dram_parameter("Wp2w1", [C, G], F16, isOutput=False)
    Ww2B_d = nc.declare_dram_parameter("Ww2B", [8 * G, 8 * G], F16, isOutput=False)
    out = nc.declare_dram_parameter("out", [NR, C], F32, isOutput=True)

    packed = nc.dram_tensor("packed", [NT2, SRE], F16)
    qpack = nc.dram_tensor("qpack", [NR, 16], F16)

    with ExitStack() as ctx:
        tc = ctx.enter_context(tile.TileContext(nc))
        consts = ctx.enter_context(tc.tile_pool(name="consts", bufs=1))
        # SBUF pools
        sb_in = ctx.enter_context(tc.tile_pool(name="sb_in", bufs=3))
        sb_t = ctx.enter_context(tc.tile_pool(name="sb_t", bufs=3))
        sb_st = ctx.enter_context(tc.tile_pool(name="sb_st", bufs=3))
        sb_sm = ctx.enter_context(tc.tile_pool(name="sb_sm", bufs=4))
        sb_g = ctx.enter_context(tc.tile_pool(name="sb_g", bufs=5))
        sb_b = ctx.enter_context(tc.tile_pool(name="sb_b", bufs=3))
        sb_w = ctx.enter_context(tc.tile_pool(name="sb_w", bufs=4))

        ident = consts.tile([P, P], F16)
        make_identity(nc, ident[:])
        epst = consts.tile([P, 1], F32)
        nc.vector.memset(epst[:], EPS)
        ones96 = consts.tile([C, 1], F16)
        nc.vector.memset(ones96[:], 1.0)
        ones12 = consts.tile([1, G], F16)
        nc.vector.memset(ones12[:], 1.0)

        def load_const(name, dram, shape):
            t = consts.tile(shape, F16, tag=name)
            nc.sync.dma_start(out=t[:], in_=dram[:])
            return t

        wq_sb = load_const("wq", Wq_d, [C, C])
        wk_sb = load_const("wk", Wk_d, [C, C])
        wv_sb = load_const("wv", Wv_d, [C, C])
        ww1_sb = load_const("ww1", Ww1_d, [C, G])
        wp1b_sb = load_const("wp1b", Wp1blk_d, [3 * S, S * C])
        mqb_sb = load_const("mqb", MqB_d, [3 * S, 3 * S])
        wp2_sb = load_const("wp2", Wp2_d, [C, C])
        wp2w1_sb = load_const("wp2w1", Wp2w1_d, [C, C // CG])
        ww2b_sb = load_const("ww2b", Ww2B_d, [8 * G, 8 * G])

        # ---------------- Phase A: packed superrow table (feature-major) ------
        # Per 512-point chunk: y=Wkc^T.kT  (LN via ssq->rstd), b0=Ww1c^T.relu(y),
        # vv=Wv^T.vT.  bundle [112,512] = [vv 96 | b0 12 | xyz 3 | rstd 1];
        # 4 PE transposes (strided cols j::4) -> point-major stg; bkey scaled
        # by rstd post-transpose; DMA to packed.
        actx = ExitStack()
        pp_y = actx.enter_context(tc.tile_pool(name="pa_y", bufs=2, space="PSUM"))
        pp_v = actx.enter_context(tc.tile_pool(name="pa_v", bufs=2, space="PSUM"))
        pp_tp = actx.enter_context(tc.tile_pool(name="pa_tp", bufs=2, space="PSUM"))
        pp_w = actx.enter_context(tc.tile_pool(name="pa_w", bufs=2, space="PSUM"))

        def chunk_fm(src_dram, w_sb, c0, with_val):
            """Returns bundle [112, 512] f16 (val|b0|xyz|rstd rows) or
            [16, 512] for the q-side (a0|xyz|rstd)."""
            xc = sb_in.tile([C, 512], F16, tag="xc")
            nc.sync.dma_start(out=xc[:], in_=src_dram[:, c0:c0 + 512])
            y = pp_y.tile([C, 512], F32, tag="y")
            nc.tensor.matmul(out=y[:], lhsT=w_sb[:], rhs=xc[:],
                             start=True, stop=True)
            rk = sb_t.tile([C, 512], F16, tag="rk")
            nc.vector.tensor_scalar_max(out=rk[:], in0=y[:], scalar1=0.0)
            sq = sb_sm.tile([C, 512], F16, tag="sq")
            nc.scalar.activation(out=sq[:], in_=y[:], func=ACTF.Square)
            wb = pp_w.tile([33, 512], F32, tag="wb")
            nc.tensor.matmul(out=wb[32:33, :], lhsT=ones96[:], rhs=sq[:],
                             start=True, stop=True)
            nc.tensor.matmul(out=wb[0:G, :], lhsT=ww1_sb[:], rhs=rk[:],
                             start=True, stop=True)
            ssqs = sb_sm.tile([1, 512], F16, tag="ssqs")
            nc.vector.tensor_copy(out=ssqs[:], in_=wb[32:33, :])
            return wb, ssqs

        for b in range(NT // 512):
            c0 = b * 512
            sr0 = c0 // 2
            bundle = sb_st.tile([112, 512], F16, tag="bun")
            nc.sync.dma_start(out=bundle[108:111, :],
                              in_=xyzT_d[:, c0:c0 + 512])
            wb, ssqs = chunk_fm(kT_d, wk_sb, c0, True)
            nc.vector.tensor_copy(out=bundle[96:108, :], in_=wb[0:G, :])
            nc.sync.dma_start(out=bundle[111:112, :], in_=ssqs[:])
            vc = sb_in.tile([C, 512], F16, tag="vc")
            nc.sync.dma_start(out=vc[:], in_=vT_d[:, c0:c0 + 512])
            vv = pp_v.tile([C, 512], F32, tag="v")
            nc.tensor.matmul(out=vv[:], lhsT=wv_sb[:], rhs=vc[:],
                             start=True, stop=True)
            nc.scalar.copy(out=bundle[0:96, :], in_=vv[:])

            tp = pp_tp.tile([P, 4, 112], F16, tag="tp")
            bview = bundle[:].rearrange("f (p j) -> f j p", j=4)
            for j in range(4):
                nc.tensor.transpose(out=tp[:, j, :], in_=bview[:, j, :],
                                    identity=ident[0:112, 0:112])
            stg = sb_st.tile([P, 2, SRE], F16, tag="stg")
            stg4 = stg[:].rearrange("p a (o x) -> p (a o) x", o=2)  # [P,4,128]
            nc.vector.tensor_copy(out=stg4[:, :, 0:112], in_=tp[:])
            nc.sync.dma_start(
                out=packed[sr0:sr0 + 256, :].rearrange("(p a) e -> p a e", a=2),
                in_=stg[:])

        # ---------------- Phase A2: qpack [aq 12 | xyz 3 | rstd] --------------
        for b in range(NR // 512):
            c0 = b * 512
            bq = sb_st.tile([16, 512], F16, tag="bq")
            nc.sync.dma_start(out=bq[12:15, :], in_=xyzsT_d[:, c0:c0 + 512])
            wb, ssqs = chunk_fm(qT_d, wq_sb, c0, False)
            nc.vector.tensor_copy(out=bq[0:G, :], in_=wb[0:G, :])
            nc.sync.dma_start(out=bq[15:16, :], in_=ssqs[:])
            tpq_full = pp_tp.tile([P, 4, 112], F16, tag="tp")
            tpq = tpq_full[:, :, 0:16]
            bqv = bq[:].rearrange("f (p j) -> f j p", j=4)
            for j in range(4):
                nc.tensor.transpose(out=tpq[:, j, :], in_=bqv[:, j, :],
                                    identity=ident[0:16, 0:16])
            qstg = sb_st.tile([P, 4, 16], F16, tag="qstg")
            nc.vector.tensor_copy(out=qstg[:], in_=tpq[:])
            nc.sync.dma_start(
                out=qpack[c0:c0 + 512, :].rearrange("(p a) e -> p a e", a=4),
                in_=qstg[:])

        actx.close()
        # PSUM pools (8 banks: 2+2+2+2)
        pp_y = ctx.enter_context(tc.tile_pool(name="pp_y", bufs=2, space="PSUM"))
        pp_v = ctx.enter_context(tc.tile_pool(name="pp_v", bufs=2, space="PSUM"))
        pp_tp = ctx.enter_context(tc.tile_pool(name="pp_tp", bufs=2, space="PSUM"))
        pp_w = ctx.enter_context(tc.tile_pool(name="pp_w", bufs=2, space="PSUM"))

        # ---------------- Phase B: per 128-point tile --------------------------
        for t in range(NR // P):
            r0 = t * P
            qp = sb_sm.tile([P, 16], F16, tag="qp")
            nc.sync.dma_start(out=qp[:], in_=qpack[r0:r0 + P, :])
            pr = sb_sm.tile([P, S], F16, tag="pr")
            nc.sync.dma_start(out=pr[:], in_=par_d[r0:r0 + P, :])
            ixt = sb_sm.tile([P, P], I16, tag="ixt")
            nc.sync.dma_start(out=ixt[:], in_=idx16_d[r0:r0 + P, :])
            Gt = sb_g.tile([P, S, SRE], F16, tag="G")
            nc.gpsimd.dma_gather(
                out_ap=Gt[:], in_ap=packed[:, :], idxs_ap=ixt[:],
                num_idxs=P * S, num_idxs_reg=P * S, elem_size=SRE,
                single_packet=False, queue_num=t % 2)
            Gpair = Gt[:].rearrange("p s (o x) -> p s o x", o=2)

            # parity-select the 16 small fields: [b0 12 | xyz 3 | ssq 1]
            prb16 = pr[:].rearrange("p (s o) -> p s o", o=1).broadcast_to([P, S, 16])
            d15 = sb_sm.tile([P, S, 16], F16, tag="d15")
            nc.vector.tensor_tensor(out=d15[:], in0=Gpair[:, :, 1, 96:112],
                                    in1=Gpair[:, :, 0, 96:112], op=ALU.subtract)
            nc.vector.tensor_tensor(out=d15[:], in0=d15[:], in1=prb16,
                                    op=ALU.mult)
            sel = sb_sm.tile([P, S, 16], F16, tag="sel")
            nc.vector.tensor_tensor(out=sel[:], in0=Gpair[:, :, 0, 96:112],
                                    in1=d15[:], op=ALU.add)
            # k-side LN scale: bks = b0 * rsqrt(ssq/C + eps)
            sdk = sb_sm.tile([P, S], F32, tag="sdk")
            nc.scalar.activation(out=sdk[:],
                                 in_=sel[:, :, 15:16].rearrange("p s o -> p (s o)"),
                                 func=ACTF.Sqrt, scale=1.0 / C, bias=epst[:])
            rkk = sb_sm.tile([P, S], F32, tag="rkk")
            nc.vector.reciprocal(out=rkk[:], in_=sdk[:])
            bks = sb_sm.tile([P, S, G], F16, tag="bks")
            nc.vector.tensor_tensor(
                out=bks[:], in0=sel[:, :, 0:12],
                in1=rkk[:].rearrange("p (s o) -> p s o", o=1)
                    .broadcast_to([P, S, G]),
                op=ALU.mult)
            # q-side LN scale: aqs = aq0 * rsqrt(ssq_q/C + eps)
            sdq = sb_sm.tile([P, 1], F32, tag="sdq")
            nc.scalar.activation(out=sdq[:], in_=qp[:, 15:16],
                                 func=ACTF.Sqrt, scale=1.0 / C, bias=epst[:])
            rkq = sb_sm.tile([P, 1], F32, tag="rkq")
            nc.vector.reciprocal(out=rkq[:], in_=sdq[:])
            aqs = sb_sm.tile([P, G], F16, tag="aqs")
            nc.vector.tensor_tensor(out=aqs[:], in0=qp[:, 0:12],
                                    in1=rkq[:].broadcast_to([P, G]),
                                    op=ALU.mult)

            # pos and its transpose
            ps = sb_sm.tile([P, S, 3], F16, tag="ps")
            nc.vector.tensor_tensor(
                out=ps[:], in0=sel[:, :, 12:15],
                in1=qp[:, 12:15].rearrange("p (o c) -> p o c", o=1)
                    .broadcast_to([P, S, 3]),
                op=ALU.subtract)
            posTp = pp_tp.tile([3 * S, P], F16, tag="tp")
            nc.tensor.transpose(out=posTp[:], in_=ps[:].rearrange("p s c -> p (s c)"),
                                identity=ident[:])
            posT = sb_t.tile([3 * S, P], F16, tag="posT")
            nc.vector.tensor_copy(out=posT[:], in_=posTp[:])

            # rstd via 3x3 Gram quadratic form: ssq = sum_c (pos@Wp1c)^2
            qf = pp_w.tile([P, 3 * S], F32, tag="w")
            nc.tensor.matmul(out=qf[:], lhsT=posT[:], rhs=mqb_sb[:],
                             start=True, stop=True)
            s2 = sb_sm.tile([P, S, 3], F16, tag="s2")
            nc.vector.tensor_tensor(
                out=s2[:], in0=ps[:],
                in1=qf[:].rearrange("p (s c) -> p s c", c=3), op=ALU.mult)
            ssqp = sb_sm.tile([P, S], F32, tag="ssqp")
            nc.vector.tensor_reduce(out=ssqp[:], in_=s2[:], axis=AX, op=ALU.add)
            sdp = sb_sm.tile([P, S], F32, tag="sdp")
            nc.scalar.activation(out=sdp[:], in_=ssqp[:], func=ACTF.Sqrt,
                                 scale=1.0 / C, bias=epst[:])
            rstd = sb_sm.tile([P, S], F32, tag="rstd")
            nc.vector.reciprocal(out=rstd[:], in_=sdp[:])
            rstdb = rstd[:].rearrange("p (s o) -> p s o", o=1)

            # pu matmuls (block-diag Wp1 stationaries) + relu -> pLT
            pLT = sb_b.tile([C, S, P], F16, tag="pLT")
            for g4 in range(4):
                puP = pp_y.tile([C, 4, P], F32, tag="y")
                for j in range(4):
                    s = g4 * 4 + j
                    nc.tensor.matmul(out=puP[:, j, :],
                                     lhsT=wp1b_sb[:, s * C:(s + 1) * C],
                                     rhs=posT[:], start=True, stop=True)
                nc.scalar.activation(out=pLT[:, g4 * 4:(g4 + 1) * 4, :],
                                     in_=puP[:], func=ACTF.Relu)

            # pwa = relu(pu) @ (Wp2@Ww1c)  [the 12-dim weight-branch pos term]
            pwaP = pp_w.tile([P, S, G], F32, tag="w")
            for s in range(S):
                nc.tensor.matmul(out=pwaP[:, s, :], lhsT=pLT[:, s, :],
                                 rhs=wp2w1_sb[:], start=True, stop=True)

            # weight branch: yt = (bkey_g - aq) + rstd*pwa ; LN_G ; relu
            # (hot tiles live in sb_w, padded to 512B so every pool rotation
            # stays 512B-aligned -- misaligned fp16 DVE ops run ~30x slower)
            yt_t = sb_w.tile([P, 256], F16, tag="yt")
            yt = yt_t[:, 0:192].rearrange("p (s g) -> p s g", g=12)
            nc.vector.tensor_tensor(
                out=yt, in0=bks[:],
                in1=aqs[:].rearrange("p (o c) -> p o c", o=1)
                    .broadcast_to([P, S, G]),
                op=ALU.subtract)
            tyr = sb_sm.tile([P, S, G], F16, tag="tyr")
            nc.vector.tensor_tensor(out=tyr[:], in0=pwaP[:],
                                    in1=rstdb.broadcast_to([P, S, G]), op=ALU.mult)
            nc.vector.tensor_tensor(out=yt, in0=yt, in1=tyr[:], op=ALU.add)
            sqg = sb_sm.tile([P, S, G], F16, tag="sqg")
            nc.scalar.activation(out=sqg[:], in_=yt, func=ACTF.Square)
            ssqg = sb_sm.tile([P, S], F32, tag="ssqg")
            nc.vector.tensor_reduce(out=ssqg[:], in_=sqg[:], axis=AX, op=ALU.add)
            sdg = sb_sm.tile([P, S], F32, tag="sdg")
            nc.scalar.activation(out=sdg[:], in_=ssqg[:], func=ACTF.Sqrt,
                                 scale=1.0 / G, bias=epst[:])
            rsg = sb_sm.tile([P, S], F32, tag="rsg")
            nc.vector.reciprocal(out=rsg[:], in_=sdg[:])
            yh_t = sb_w.tile([P, 256], F16, tag="yh")
            yh = yh_t[:, 0:192].rearrange("p (s g) -> p s g", g=12)
            nc.vector.tensor_tensor(
                out=yh, in0=yt,
                in1=rsg[:].rearrange("p (s o) -> p s o", o=1)
                    .broadcast_to([P, S, G]),
                op=ALU.mult)
            nc.vector.tensor_scalar_max(out=yh, in0=yh, scalar1=0.0)

            # z = yh @ kron(I8, Ww2) -> e = exp(z)
            yflat = yh_t[:, 0:192]
            yT = sb_t.tile([C, 2, P], F16, tag="yT")
            for h in range(2):
                yhTp = pp_tp.tile([C, P], F16, tag="tp")
                nc.tensor.transpose(out=yhTp[:], in_=yflat[:, h * C:(h + 1) * C],
                                    identity=ident[:])
                nc.scalar.copy(out=yT[:, h, :], in_=yhTp[:])
            zP = pp_w.tile([P, 2, C], F32, tag="w")
            for h in range(2):
                nc.tensor.matmul(out=zP[:, h, :], lhsT=yT[:, h, :],
                                 rhs=ww2b_sb[:], start=True, stop=True)
            e_t = sb_w.tile([P, 256], F16, tag="e")
            e = e_t[:, 0:192].rearrange("p (s g) -> p s g", g=12)
            nc.scalar.activation(out=e_t[:, 0:192],
                                 in_=zP[:].rearrange("p a c -> p (a c)"),
                                 func=ACTF.Exp)
            es = sb_sm.tile([P, G], F32, tag="es")
            nc.vector.tensor_reduce(out=es[:], in_=e.rearrange("p s g -> p g s"),
                                    axis=AX, op=ALU.add)
            rq = sb_sm.tile([P, G], F32, tag="rq")
            nc.vector.reciprocal(out=rq[:], in_=es[:])

            # parity-masked weights for the value field + u for the peb field
            ep_t = sb_w.tile([P, 512], F16, tag="ep")
            ep = ep_t[:, 0:384].rearrange("p (s o g) -> p s o g", o=2, g=12)
            nc.vector.tensor_tensor(
                out=ep[:, :, 1, :], in0=e,
                in1=pr[:].rearrange("p (s o) -> p s o", o=1).broadcast_to([P, S, G]),
                op=ALU.mult)
            nc.vector.tensor_tensor(out=ep[:, :, 0, :], in0=e,
                                    in1=ep[:, :, 1, :], op=ALU.subtract)
            u_t = sb_w.tile([P, 256], F16, tag="u")
            u = u_t[:, 0:192].rearrange("p (s g) -> p s g", g=12)
            nc.vector.tensor_tensor(out=u, in0=e,
                                    in1=rstdb.broadcast_to([P, S, G]), op=ALU.mult)

            # weighted sums: macc = sum of e'*val(parity) and u*pebraw
            macc = sb_b.tile([P, S, C], F16, tag="macc")
            nc.vector.tensor_tensor(
                out=macc[:].rearrange("p s (g o) -> p s g o", o=CG),
                in0=Gpair[:, :, 0, 0:96].rearrange("p s (g o) -> p s g o", o=CG),
                in1=ep[:, :, 0, :].rearrange("p s (g o) -> p s g o", o=1)
                    .broadcast_to([P, S, G, CG]),
                op=ALU.mult)
            m1b = sb_b.tile([P, S, C], F16, tag="m1b")
            nc.vector.tensor_tensor(
                out=m1b[:].rearrange("p s (g o) -> p s g o", o=CG),
                in0=Gpair[:, :, 1, 0:96].rearrange("p s (g o) -> p s g o", o=CG),
                in1=ep[:, :, 1, :].rearrange("p s (g o) -> p s g o", o=1)
                    .broadcast_to([P, S, G, CG]),
                op=ALU.mult)
            nc.vector.tensor_tensor(out=macc[:], in0=macc[:], in1=m1b[:],
                                    op=ALU.add)

            m2 = sb_b.tile([P, S, C], F16, tag="m2")
            for g4 in range(4):
                pebP = pp_v.tile([P, 4, C], F32, tag="v")
                for j in range(4):
                    s = g4 * 4 + j
                    nc.tensor.matmul(out=pebP[:, j, :], lhsT=pLT[:, s, :],
                                     rhs=wp2_sb[:], start=True, stop=True)
                nc.vector.tensor_tensor(
                    out=m2[:, g4 * 4:(g4 + 1) * 4, :]
                        .rearrange("p s (g o) -> p s g o", o=CG),
                    in0=pebP[:].rearrange("p s (g o) -> p s g o", o=CG),
                    in1=u[:, g4 * 4:(g4 + 1) * 4, :]
                        .rearrange("p s (g o) -> p s g o", o=1)
                        .broadcast_to([P, 4, G, CG]),
                    op=ALU.mult)
            nc.vector.tensor_tensor(out=macc[:], in0=macc[:], in1=m2[:],
                                    op=ALU.add)

            for hw_ in (8, 4, 2, 1):
                nc.vector.tensor_tensor(out=macc[:, 0:hw_, :],
                                        in0=macc[:, 0:hw_, :],
                                        in1=macc[:, hw_:2 * hw_, :], op=ALU.add)
            fo = sb_sm.tile([P, C], F32, tag="fo")
            nc.vector.tensor_tensor(
                out=fo[:].rearrange("p (g o) -> p g o", o=CG),
                in0=macc[:, 0, :].rearrange("p (g o) -> p g o", o=CG),
                in1=rq[:].rearrange("p (g o) -> p g o", o=1).broadcast_to([P, G, CG]),
                op=ALU.mult)
            nc.sync.dma_start(out=out[r0:r0 + P, :], in_=fo[:])

    nc.finalize()
    return nc


def _center(W):
    """Remove the mean over the output axis (last)."""
    W = np.asarray(W, np.float64)
    return (W - W.mean(axis=-1, keepdims=True)).astype(np.float32)


def _prep_host(q, k, v, xyz, reference_index,
               Wq, bq, gq, betaq, Wk, bk, gk, betak, Wv, bv,
               Wp1, bp1, gp, betap, Wp2, bp2, Ww1, bw1, gw, betaw, Ww2, bw2,
               n_cores):
    for name, arr, val in [
        ("bq", bq, 0), ("gq", gq, 1), ("betaq", betaq, 0),
        ("bk", bk, 0), ("gk", gk, 1), ("betak", betak, 0),
        ("bv", bv, 0), ("bp1", bp1, 0), ("gp", gp, 1), ("betap", betap, 0),
        ("bp2", bp2, 0), ("bw1", bw1, 0), ("gw", gw, 1), ("betaw", betaw, 0),
        ("bw2", bw2, 0),
    ]:
        if not np.allclose(np.asarray(arr), val, atol=1e-6):
            raise NotImplementedError(f"non-trivial {name} not supported")

    N = q.shape[0]
    NR = ((N // n_cores) + 511) // 512 * 512
    NT = (N + 1023) // 1024 * 1024

    def padT(a, rows, dtype=np.float16):
        out = np.zeros((rows, a.shape[1]), dtype=dtype)
        out[:a.shape[0]] = np.asarray(a)
        return out

    kT = np.ascontiguousarray(padT(k, NT).T)      # [C, NT] f16
    vT = np.ascontiguousarray(padT(v, NT).T)
    xyzT = np.ascontiguousarray(padT(xyz, NT).T)  # [3, NT]

    Wq32 = _center(Wq)
    Wk32 = _center(Wk)
    Ww1c = _center(Ww1)
    Wp1c = _center(Wp1)                       # [3, C]
    Wp1c16 = Wp1c.astype(np.float16)
    M3 = (Wp1c16.astype(np.float32) @ Wp1c16.astype(np.float32).T)  # [3,3]
    MqB = np.kron(np.eye(S, dtype=np.float32), M3)                  # [48,48]
    Wp1blk = np.zeros((3 * S, S * C), np.float32)
    for s in range(S):
        Wp1blk[3 * s:3 * s + 3, s * C:(s + 1) * C] = Wp1c
    weights = {
        "Wqc": Wq32.astype(np.float16),
        "Wkc": Wk32.astype(np.float16),
        "Wv": np.asarray(Wv, np.float32).astype(np.float16),
        "Ww1c": Ww1c.astype(np.float16),
        "Wp1blk": Wp1blk.astype(np.float16),
        "MqB": MqB.astype(np.float16),
        "Wp2": np.asarray(Wp2, np.float32).astype(np.float16),
        "Wp2w1": (np.asarray(Wp2, np.float32) @ Ww1c).astype(np.float16),
        "Ww2B": np.kron(np.eye(8, dtype=np.float32),
                        np.asarray(Ww2, np.float32)).astype(np.float16),
    }

    per_core = N // n_cores
    assert per_core * n_cores == N
    ref = np.asarray(reference_index, np.int64)
    in_maps = []
    for i in range(n_cores):
        lo, hi = i * per_core, (i + 1) * per_core
        rsl = ref[lo:hi]                       # [per_core, S]
        nt_tiles = NR // P
        # idx16[t*128+p-like rows, 128]: per 128-row tile, int16 half-indices
        # ordered so gather pair j = s*128+p -> idxs[(j%16) within 16-part
        # block replicated 8x, j//16].
        idx16 = np.zeros((NR, P), np.int16)
        par = np.zeros((NR, S), np.float16)
        half = np.zeros((NR, S), np.int16)
        half[:per_core] = (rsl >> 1).astype(np.int16)
        par[:per_core] = (rsl & 1).astype(np.float16)
        for t in range(nt_tiles):
            blk = half[t * P:(t + 1) * P]          # [128, S]
            lin = blk.T.reshape(-1)                # j = s*128+p
            i16 = lin.reshape(P, 16).T             # [16, 128]
            idx16[t * P:(t + 1) * P] = np.tile(i16, (8, 1))
        m = {
            "kT": kT, "vT": vT, "xyzT": xyzT,
            "qT": np.ascontiguousarray(padT(q[lo:hi], NR).T),
            "xyzsT": np.ascontiguousarray(padT(xyz[lo:hi], NR).T),
            "idx16": idx16,
            "par": par,
        }
        m.update(weights)
        in_maps.append(m)
    return in_maps, NR, NT, per_core


_CACHE = {}


def kernel(**inputs):
    n_cores = 8
    in_maps, NR, NT, per_core = _prep_host(n_cores=n_cores, **inputs)
    key = (NR, NT)
    if key not in _CACHE:
        _CACHE[key] = _build(NR, NT)
    nc = _CACHE[key]
    res = run_bass_kernel_spmd(nc, in_maps, list(range(n_cores)))
    outs = [res.results[i]["out"][:per_core] for i in range(n_cores)]
    return np.ascontiguousarray(np.concatenate(outs, axis=0), dtype=np.float32)


# revision 4
# speedup vs baseline: 1.3184x; 1.0033x over previous
"""GroupedVectorAttention Trainium2 kernel v3 (8-core SPMD, data-parallel).

v3 = v2 Phase B (dma_gather superrow design, unchanged) + rewritten Phase A:
the k/v/q projections run feature-major with HOST-TRANSPOSED inputs
(kT/vT/qT/xyzT), so each 512-row chunk needs only 4 weight-stationary
matmuls + 4 PE transposes (for the point-major packed-table write) instead
of v2's 12 transposes + 12 data-stationary matmuls + 12 DVE copies.
The LN rstd rides the transpose as bundle row 111 and is applied to the
bkey fields point-major afterwards.
"""

import numpy as np
from contextlib import ExitStack

import concourse.bass as bass
import concourse.bacc as bacc
import concourse.tile as tile
from concourse import mybir
from concourse.bass_utils import run_bass_kernel_spmd
from concourse.masks import make_identity

# ---------------------------------------------------------------------------
# Workaround: this walrus build rejects PE Matmult instructions carrying more
# than one semaphore wait ("Too many sync wait commands").  Split extra waits
# onto NoOp carrier instructions on the same engine queue, placed immediately
# before the matmul, right after Tile's wait-assignment pass.
_orig_postorder = tile.postorder_instruction_blocks
_nop_ctr = [0]


def _split_pe_waits(ordered, *args, **kwargs):
    for bb, insts in list(ordered.items()):
        out = []
        for inst in insts:
            si = getattr(inst, "sync_info", None)
            if (si is not None and si.on_wait and len(si.on_wait) > 1
                    and inst.engine != mybir.EngineType.Pool):
                waits = list(si.on_wait)
                for w in waits[:-1]:
                    _nop_ctr[0] += 1
                    nop = mybir.InstNoOp(name=f"nopw-{_nop_ctr[0]}")
                    nop.engine = inst.engine
                    nop.sync_info = mybir.SyncInfo(on_wait=[w], on_update=[])
                    out.append(nop)
                inst.sync_info = mybir.SyncInfo(
                    on_wait=[waits[-1]], on_update=list(si.on_update)
                )
            out.append(inst)
        ordered[bb] = out
    return _orig_postorder(ordered, *args, **kwargs)


tile.postorder_instruction_blocks = _split_pe_waits

from concourse.vector_clock import ScopedClock as _ScopedClock


def _patched_drain_and_barrier(self, tick_clock, wait_clock):
    probe = self.nc.sync.nop(nofuse=True)
    wait_clock.add_sem_waits(
        probe.ins, _ScopedClock({None: tick_clock.global_clock})
    )
    si = probe.ins.sync_info
    if si is not None and si.on_wait and len(si.on_wait) > 1:
        waits = list(si.on_wait)
        probe.ins.sync_info = mybir.SyncInfo(
            on_wait=waits[:1], on_update=list(si.on_update)
        )
        for w in waits[1:]:
            n2 = self.nc.sync.nop(nofuse=True)
            n2.ins.sync_info = mybir.SyncInfo(on_wait=[w], on_update=[])
    self.nc.sync.drain()
    self.nc.all_engine_barrier()
    popped = self.nc._tile_sem_poison_stack.pop()
    assert popped is self._sem_poison
    self.nc.clear_and_free_semaphores(list(self.sems.allocated().values()))
    self.nc.all_engine_barrier()


tile.TileContext._drain_and_barrier = _patched_drain_and_barrier

P = 128
C = 96
G = 12
S = 16
CG = C // G  # 8
EPS = 1e-5
SRE = 256        # fp16 elems per 512B superrow: [row0 112|pad16|row1 112|pad16]
F16 = mybir.dt.float16
F32 = mybir.dt.float32
I16 = mybir.dt.int16
AX = mybir.AxisListType.X
ALU = mybir.AluOpType
ACTF = mybir.ActivationFunctionType


def _build(NR, NT, debug=False):
    """Per-core Bacc kernel. NR = padded rows per core (mult of 512),
    NT = padded table rows (mult of 512)."""
    assert NR % 512 == 0 and NT % 1024 == 0
    NT2 = NT // 2
    nc = bacc.Bacc(dynamic_dma_scratch_size=65536, num_swdge_queues=2,
                   num_devices=8)

    NTS = NT // 8
    kT_d = nc.declare_dram_parameter("kTs", [C, NTS], F16, isOutput=False)
    vT_d = nc.declare_dram_parameter("vTs", [C, NTS], F16, isOutput=False)
    xyzT_d = nc.declare_dram_parameter("xyzTs", [3, NTS], F16, isOutput=False)
    qT_d = nc.declare_dram_parameter("qT", [C, NR], F16, isOutput=False)
    xyzsT_d = nc.declare_dram_parameter("xyzsT", [3, NR], F16, isOutput=False)
    idx16_d = nc.declare_dram_parameter("idx16", [NR, P], I16, isOutput=False)
    par_d = nc.declare_dram_parameter("par", [NR, S], F16, isOutput=False)
    Wq_d = nc.declare_dram_parameter("Wqc", [C, C], F16, isOutput=False)
    Wk_d = nc.declare_dram_parameter("Wkc", [C, C], F16, isOutput=False)
    Wv_d = nc.declare_dram_parameter("Wv", [C, C], F16, isOutput=False)
    Ww1_d = nc.declare_dram_parameter("Ww1c", [C, G], F16, isOutput=False)
    Wp1blk_d = nc.declare_dram_parameter("Wp1blk", [3 * S, S * C], F16,
                                         isOutput=False)
    MqB_d = nc.declare_dram_parameter("MqB", [3 * S, 3 * S], F16, isOutput=False)
    Wp2_d = nc.declare_dram_parameter("Wp2", [C, C], F16, isOutput=False)
    Wp2w1_d = nc.declare_dram_parameter("Wp2w1", [C, G], F16, isOutput=False)
    Ww2B_d = nc.declare_dram_parameter("Ww2B", [8 * G, 8 * G], F16, isOutput=False)
    out = nc.declare_dram_parameter("out", [NR, C], F32, isOutput=True)

    packed = nc.dram_tensor("packed", [NT2, SRE], F16)
    packed_l = nc.dram_tensor("packed_l", [NTS // 2, SRE], F16)
    qpack = nc.dram_tensor("qpack", [NR, 16], F16)

    with ExitStack() as ctx:
        tc = ctx.enter_context(tile.TileContext(nc))
        consts = ctx.enter_context(tc.tile_pool(name="consts", bufs=1))
        # SBUF pools
        sb_in = ctx.enter_context(tc.tile_pool(name="sb_in", bufs=3))
        sb_t = ctx.enter_context(tc.tile_pool(name="sb_t", bufs=3))
        sb_st = ctx.enter_context(tc.tile_pool(name="sb_st", bufs=3))
        sb_sm = ctx.enter_context(tc.tile_pool(name="sb_sm", bufs=4))
        sb_g = ctx.enter_context(tc.tile_pool(name="sb_g", bufs=5))
        sb_b = ctx.enter_context(tc.tile_pool(name="sb_b", bufs=3))
        sb_w = ctx.enter_context(tc.tile_pool(name="sb_w", bufs=4))

        ident = consts.tile([P, P], F16)
        make_identity(nc, ident[:])
        epst = consts.tile([P, 1], F32)
        nc.vector.memset(epst[:], EPS)
        ones96 = consts.tile([C, 1], F16)
        nc.vector.memset(ones96[:], 1.0)
        ones12 = consts.tile([1, G], F16)
        nc.vector.memset(ones12[:], 1.0)

        def load_const(name, dram, shape):
            t = consts.tile(shape, F16, tag=name)
            nc.sync.dma_start(out=t[:], in_=dram[:])
            return t

        wq_sb = load_const("wq", Wq_d, [C, C])
        wk_sb = load_const("wk", Wk_d, [C, C])
        wv_sb = load_const("wv", Wv_d, [C, C])
        ww1_sb = load_const("ww1", Ww1_d, [C, G])
        wp1b_sb = load_const("wp1b", Wp1blk_d, [3 * S, S * C])
        mqb_sb = load_const("mqb", MqB_d, [3 * S, 3 * S])
        wp2_sb = load_const("wp2", Wp2_d, [C, C])
        wp2w1_sb = load_const("wp2w1", Wp2w1_d, [C, C // CG])
        ww2b_sb = load_const("ww2b", Ww2B_d, [8 * G, 8 * G])

        # ---------------- Phase A: packed superrow table (feature-major) ------
        # Per 512-point chunk: y=Wkc^T.kT  (LN via ssq->rstd), b0=Ww1c^T.relu(y),
        # vv=Wv^T.vT.  bundle [112,512] = [vv 96 | b0 12 | xyz 3 | rstd 1];
        # 4 PE transposes (strided cols j::4) -> point-major stg; bkey scaled
        # by rstd post-transpose; DMA to packed.
        actx = ExitStack()
        pp_y = actx.enter_context(tc.tile_pool(name="pa_y", bufs=2, space="PSUM"))
        pp_v = actx.enter_context(tc.tile_pool(name="pa_v", bufs=2, space="PSUM"))
        pp_tp = actx.enter_context(tc.tile_pool(name="pa_tp", bufs=2, space="PSUM"))
        pp_w = actx.enter_context(tc.tile_pool(name="pa_w", bufs=2, space="PSUM"))

        def chunk_fm(src_dram, w_sb, c0, with_val):
            """Returns bundle [112, 512] f16 (val|b0|xyz|rstd rows) or
            [16, 512] for the q-side (a0|xyz|rstd)."""
            xc = sb_in.tile([C, 512], F16, tag="xc")
            nc.sync.dma_start(out=xc[:], in_=src_dram[:, c0:c0 + 512])
            y = pp_y.tile([C, 512], F32, tag="y")
            nc.tensor.matmul(out=y[:], lhsT=w_sb[:], rhs=xc[:],
                             start=True, stop=True)
            rk = sb_t.tile([C, 512], F16, tag="rk")
            nc.vector.tensor_scalar_max(out=rk[:], in0=y[:], scalar1=0.0)
            sq = sb_sm.tile([C, 512], F16, tag="sq")
            nc.scalar.activation(out=sq[:], in_=y[:], func=ACTF.Square)
            wb = pp_w.tile([33, 512], F32, tag="wb")
            nc.tensor.matmul(out=wb[32:33, :], lhsT=ones96[:], rhs=sq[:],
                             start=True, stop=True)
            nc.tensor.matmul(out=wb[0:G, :], lhsT=ww1_sb[:], rhs=rk[:],
                             start=True, stop=True)
            ssqs = sb_sm.tile([1, 512], F16, tag="ssqs")
            nc.vector.tensor_copy(out=ssqs[:], in_=wb[32:33, :])
            return wb, ssqs

        for b in range(NTS // 512):
            c0 = b * 512
            sr0 = c0 // 2
            bundle = sb_st.tile([112, 512], F16, tag="bun")
            nc.sync.dma_start(out=bundle[108:111, :],
                              in_=xyzT_d[:, c0:c0 + 512])
            wb, ssqs = chunk_fm(kT_d, wk_sb, c0, True)
            nc.vector.tensor_copy(out=bundle[96:108, :], in_=wb[0:G, :])
            nc.sync.dma_start(out=bundle[111:112, :], in_=ssqs[:])
            vc = sb_in.tile([C, 512], F16, tag="vc")
            nc.sync.dma_start(out=vc[:], in_=vT_d[:, c0:c0 + 512])
            vv = pp_v.tile([C, 512], F32, tag="v")
            nc.tensor.matmul(out=vv[:], lhsT=wv_sb[:], rhs=vc[:],
                             start=True, stop=True)
            nc.scalar.copy(out=bundle[0:96, :], in_=vv[:])

            tp = pp_tp.tile([P, 4, 112], F16, tag="tp")
            bview = bundle[:].rearrange("f (p j) -> f j p", j=4)
            for j in range(4):
                nc.tensor.transpose(out=tp[:, j, :], in_=bview[:, j, :],
                                    identity=ident[0:112, 0:112])
            stg = sb_st.tile([P, 2, SRE], F16, tag="stg")
            stg4 = stg[:].rearrange("p a (o x) -> p (a o) x", o=2)  # [P,4,128]
            nc.vector.tensor_copy(out=stg4[:, :, 0:112], in_=tp[:])
            nc.sync.dma_start(
                out=packed_l[sr0:sr0 + 256, :].rearrange("(p a) e -> p a e", a=2),
                in_=stg[:])

        nc.gpsimd.collective_compute(
            "AllGather", mybir.AluOpType.bypass,
            replica_groups=[list(range(8))],
            ins=[packed_l.ap().opt()], outs=[packed.ap().opt()])

        # ---------------- Phase A2: qpack [aq 12 | xyz 3 | rstd] --------------
        for b in range(NR // 512):
            c0 = b * 512
            bq = sb_st.tile([16, 512], F16, tag="bq")
            nc.sync.dma_start(out=bq[12:15, :], in_=xyzsT_d[:, c0:c0 + 512])
            wb, ssqs = chunk_fm(qT_d, wq_sb, c0, False)
            nc.vector.tensor_copy(out=bq[0:G, :], in_=wb[0:G, :])
            nc.sync.dma_start(out=bq[15:16, :], in_=ssqs[:])
            tpq_full = pp_tp.tile([P, 4, 112], F16, tag="tp")
            tpq = tpq_full[:, :, 0:16]
            bqv = bq[:].rearrange("f (p j) -> f j p", j=4)
            for j in range(4):
                nc.tensor.transpose(out=tpq[:, j, :], in_=bqv[:, j, :],
                                    identity=ident[0:16, 0:16])
            qstg = sb_st.tile([P, 4, 16], F16, tag="qstg")
            nc.vector.tensor_copy(out=qstg[:], in_=tpq[:])
            nc.sync.dma_start(
                out=qpack[c0:c0 + 512, :].rearrange("(p a) e -> p a e", a=4),
                in_=qstg[:])

        actx.close()
        # PSUM pools (8 banks: 2+2+2+2)
        pp_y = ctx.enter_context(tc.tile_pool(name="pp_y", bufs=2, space="PSUM"))
        pp_v = ctx.enter_context(tc.tile_pool(name="pp_v", bufs=2, space="PSUM"))
        pp_tp = ctx.enter_context(tc.tile_pool(name="pp_tp", bufs=2, space="PSUM"))
        pp_w = ctx.enter_context(tc.tile_pool(name="pp_w", bufs=2, space="PSUM"))

        # ---------------- Phase B: per 128-point tile --------------------------
        for t in range(NR // P):
            r0 = t * P
            qp = sb_sm.tile([P, 16], F16, tag="qp")
            nc.sync.dma_start(out=qp[:], in_=qpack[r0:r0 + P, :])
            pr = sb_sm.tile([P, S], F16, tag="pr")
            nc.sync.dma_start(out=pr[:], in_=par_d[r0:r0 + P, :])
            ixt = sb_sm.tile([P, P], I16, tag="ixt")
            nc.sync.dma_start(out=ixt[:], in_=idx16_d[r0:r0 + P, :])
            Gt = sb_g.tile([P, S, SRE], F16, tag="G")
            nc.gpsimd.dma_gather(
                out_ap=Gt[:], in_ap=packed[:, :], idxs_ap=ixt[:],
                num_idxs=P * S, num_idxs_reg=P * S, elem_size=SRE,
                single_packet=False, queue_num=t % 2)
            Gpair = Gt[:].rearrange("p s (o x) -> p s o x", o=2)

            # parity-select the 16 small fields: [b0 12 | xyz 3 | ssq 1]
            prb16 = pr[:].rearrange("p (s o) -> p s o", o=1).broadcast_to([P, S, 16])
            d15 = sb_sm.tile([P, S, 16], F16, tag="d15")
            nc.vector.tensor_tensor(out=d15[:], in0=Gpair[:, :, 1, 96:112],
                                    in1=Gpair[:, :, 0, 96:112], op=ALU.subtract)
            nc.vector.tensor_tensor(out=d15[:], in0=d15[:], in1=prb16,
                                    op=ALU.mult)
            sel = sb_sm.tile([P, S, 16], F16, tag="sel")
            nc.vector.tensor_tensor(out=sel[:], in0=Gpair[:, :, 0, 96:112],
                                    in1=d15[:], op=ALU.add)
            # k-side LN scale: bks = b0 * rsqrt(ssq/C + eps)
            sdk = sb_sm.tile([P, S], F32, tag="sdk")
            nc.scalar.activation(out=sdk[:],
                                 in_=sel[:, :, 15:16].rearrange("p s o -> p (s o)"),
                                 func=ACTF.Sqrt, scale=1.0 / C, bias=epst[:])
            rkk = sb_sm.tile([P, S], F32, tag="rkk")
            nc.vector.reciprocal(out=rkk[:], in_=sdk[:])
            bks = sb_sm.tile([P, S, G], F16, tag="bks")
            nc.vector.tensor_tensor(
                out=bks[:], in0=sel[:, :, 0:12],
                in1=rkk[:].rearrange("p (s o) -> p s o", o=1)
                    .broadcast_to([P, S, G]),
                op=ALU.mult)
            # q-side LN scale: aqs = aq0 * rsqrt(ssq_q/C + eps)
            sdq = sb_sm.tile([P, 1], F32, tag="sdq")
            nc.scalar.activation(out=sdq[:], in_=qp[:, 15:16],
                                 func=ACTF.Sqrt, scale=1.0 / C, bias=epst[:])
            rkq = sb_sm.tile([P, 1], F32, tag="rkq")
            nc.vector.reciprocal(out=rkq[:], in_=sdq[:])
            aqs = sb_sm.tile([P, G], F16, tag="aqs")
            nc.vector.tensor_tensor(out=aqs[:], in0=qp[:, 0:12],
                                    in1=rkq[:].broadcast_to([P, G]),
                                    op=ALU.mult)

            # pos and its transpose
            ps = sb_sm.tile([P, S, 3], F16, tag="ps")
            nc.vector.tensor_tensor(
                out=ps[:], in0=sel[:, :, 12:15],
                in1=qp[:, 12:15].rearrange("p (o c) -> p o c", o=1)
                    .broadcast_to([P, S, 3]),
                op=ALU.subtract)
            posTp = pp_tp.tile([3 * S, P], F16, tag="tp")
            nc.tensor.transpose(out=posTp[:], in_=ps[:].rearrange("p s c -> p (s c)"),
                                identity=ident[:])
            posT = sb_t.tile([3 * S, P], F16, tag="posT")
            nc.vector.tensor_copy(out=posT[:], in_=posTp[:])

            # rstd via 3x3 Gram quadratic form: ssq = sum_c (pos@Wp1c)^2
            qf = pp_w.tile([P, 3 * S], F32, tag="w")
            nc.tensor.matmul(out=qf[:], lhsT=posT[:], rhs=mqb_sb[:],
                             start=True, stop=True)
            s2 = sb_sm.tile([P, S, 3], F16, tag="s2")
            nc.vector.tensor_tensor(
                out=s2[:], in0=ps[:],
                in1=qf[:].rearrange("p (s c) -> p s c", c=3), op=ALU.mult)
            ssqp = sb_sm.tile([P, S], F32, tag="ssqp")
            nc.vector.tensor_reduce(out=ssqp[:], in_=s2[:], axis=AX, op=ALU.add)
            sdp = sb_sm.tile([P, S], F32, tag="sdp")
            nc.scalar.activation(out=sdp[:], in_=ssqp[:], func=ACTF.Sqrt,
                                 scale=1.0 / C, bias=epst[:])
            rstd = sb_sm.tile([P, S], F32, tag="rstd")
            nc.vector.reciprocal(out=rstd[:], in_=sdp[:])
            rstdb = rstd[:].rearrange("p (s o) -> p s o", o=1)

            # pu matmuls (block-diag Wp1 stationaries) + relu -> pLT
            pLT = sb_b.tile([C, S, P], F16, tag="pLT")
            for g4 in range(4):
                puP = pp_y.tile([C, 4, P], F32, tag="y")
                for j in range(4):
                    s = g4 * 4 + j
                    nc.tensor.matmul(out=puP[:, j, :],
                                     lhsT=wp1b_sb[:, s * C:(s + 1) * C],
                                     rhs=posT[:], start=True, stop=True)
                nc.scalar.activation(out=pLT[:, g4 * 4:(g4 + 1) * 4, :],
                                     in_=puP[:], func=ACTF.Relu)

            # pwa = relu(pu) @ (Wp2@Ww1c)  [the 12-dim weight-branch pos term]
            pwaP = pp_w.tile([P, S, G], F32, tag="w")
            for s in range(S):
                nc.tensor.matmul(out=pwaP[:, s, :], lhsT=pLT[:, s, :],
                                 rhs=wp2w1_sb[:], start=True, stop=True)

            # weight branch: yt = (bkey_g - aq) + rstd*pwa ; LN_G ; relu
            # (hot tiles live in sb_w, padded to 512B so every pool rotation
            # stays 512B-aligned -- misaligned fp16 DVE ops run ~30x slower)
            yt_t = sb_w.tile([P, 256], F16, tag="yt")
            yt = yt_t[:, 0:192].rearrange("p (s g) -> p s g", g=12)
            nc.vector.tensor_tensor(
                out=yt, in0=bks[:],
                in1=aqs[:].rearrange("p (o c) -> p o c", o=1)
                    .broadcast_to([P, S, G]),
                op=ALU.subtract)
            tyr = sb_sm.tile([P, S, G], F16, tag="tyr")
            nc.vector.tensor_tensor(out=tyr[:], in0=pwaP[:],
                                    in1=rstdb.broadcast_to([P, S, G]), op=ALU.mult)
            nc.vector.tensor_tensor(out=yt, in0=yt, in1=tyr[:], op=ALU.add)
            sqg = sb_sm.tile([P, S, G], F16, tag="sqg")
            nc.scalar.activation(out=sqg[:], in_=yt, func=ACTF.Square)
            ssqg = sb_sm.tile([P, S], F32, tag="ssqg")
            nc.vector.tensor_reduce(out=ssqg[:], in_=sqg[:], axis=AX, op=ALU.add)
            sdg = sb_sm.tile([P, S], F32, tag="sdg")
            nc.scalar.activation(out=sdg[:], in_=ssqg[:], func=ACTF.Sqrt,
                                 scale=1.0 / G, bias=epst[:])
            rsg = sb_sm.tile([P, S], F32, tag="rsg")
            nc.vector.reciprocal(out=rsg[:], in_=sdg[:])
            # yh = relu(yt * rsg) = rsg * relu(yt)  (rsg > 0), and
            # kron(I8, Ww2) is block-diagonal in s, so rsg factors through
            # the z matmul: z = rsg * (relu(yt) @ Ww2B).  This keeps the
            # big [P,S,G] mult+relu off the DVE (they hit a ~7.5us SBUF
            # port-contention pathology there).
            yh_t = sb_w.tile([P, 256], F16, tag="yh")
            nc.scalar.activation(out=yh_t[:, 0:192],
                                 in_=yt_t[:, 0:192], func=ACTF.Relu)

            # z~ = relu(yt) @ kron(I8, Ww2); z = z~ * rsg; e = exp(z)
            yflat = yh_t[:, 0:192]
            yT = sb_t.tile([C, 2, P], F16, tag="yT")
            for h in range(2):
                yhTp = pp_tp.tile([C, P], F16, tag="tp")
                nc.tensor.transpose(out=yhTp[:], in_=yflat[:, h * C:(h + 1) * C],
                                    identity=ident[:])
                nc.scalar.copy(out=yT[:, h, :], in_=yhTp[:])
            zP = pp_w.tile([P, 2, C], F32, tag="w")
            for h in range(2):
                nc.tensor.matmul(out=zP[:, h, :], lhsT=yT[:, h, :],
                                 rhs=ww2b_sb[:], start=True, stop=True)
            zr_t = sb_w.tile([P, 256], F16, tag="zr")
            nc.vector.tensor_tensor(
                out=zr_t[:, 0:192].rearrange("p (h s g) -> p h s g", h=2, g=12),
                in0=zP[:].rearrange("p h (s g) -> p h s g", g=12),
                in1=rsg[:].rearrange("p (h s) -> p h s", h=2)
                    .rearrange("p h (s o) -> p h s o", o=1)
                    .broadcast_to([P, 2, 8, G]),
                op=ALU.mult)
            e_t = sb_w.tile([P, 256], F16, tag="e")
            e = e_t[:, 0:192].rearrange("p (s g) -> p s g", g=12)
            nc.scalar.activation(out=e_t[:, 0:192],
                                 in_=zr_t[:, 0:192],
                                 func=ACTF.Exp)
            es = sb_sm.tile([P, G], F32, tag="es")
            nc.vector.tensor_reduce(out=es[:], in_=e.rearrange("p s g -> p g s"),
                                    axis=AX, op=ALU.add)
            rq = sb_sm.tile([P, G], F32, tag="rq")
            nc.vector.reciprocal(out=rq[:], in_=es[:])

            # parity-masked weights for the value field + u for the peb field
            ep_t = sb_w.tile([P, 512], F16, tag="ep")
            ep = ep_t[:, 0:384].rearrange("p (s o g) -> p s o g", o=2, g=12)
            nc.vector.tensor_tensor(
                out=ep[:, :, 1, :], in0=e,
                in1=pr[:].rearrange("p (s o) -> p s o", o=1).broadcast_to([P, S, G]),
                op=ALU.mult)
            nc.vector.tensor_tensor(out=ep[:, :, 0, :], in0=e,
                                    in1=ep[:, :, 1, :], op=ALU.subtract)
            u_t = sb_w.tile([P, 256], F16, tag="u")
            u = u_t[:, 0:192].rearrange("p (s g) -> p s g", g=12)
            nc.vector.tensor_tensor(out=u, in0=e,
                                    in1=rstdb.broadcast_to([P, S, G]), op=ALU.mult)

            # weighted sums: macc = sum of e'*val(parity) and u*pebraw
            macc = sb_b.tile([P, S, C], F16, tag="macc")
            nc.vector.tensor_tensor(
                out=macc[:].rearrange("p s (g o) -> p s g o", o=CG),
                in0=Gpair[:, :, 0, 0:96].rearrange("p s (g o) -> p s g o", o=CG),
                in1=ep[:, :, 0, :].rearrange("p s (g o) -> p s g o", o=1)
                    .broadcast_to([P, S, G, CG]),
                op=ALU.mult)
            m1b = sb_b.tile([P, S, C], F16, tag="m1b")
            nc.vector.tensor_tensor(
                out=m1b[:].rearrange("p s (g o) -> p s g o", o=CG),
                in0=Gpair[:, :, 1, 0:96].rearrange("p s (g o) -> p s g o", o=CG),
                in1=ep[:, :, 1, :].rearrange("p s (g o) -> p s g o", o=1)
                    .broadcast_to([P, S, G, CG]),
                op=ALU.mult)
            nc.vector.tensor_tensor(out=macc[:], in0=macc[:], in1=m1b[:],
                                    op=ALU.add)

            m2 = sb_b.tile([P, S, C], F16, tag="m2")
            for g4 in range(4):
                pebP = pp_v.tile([P, 4, C], F32, tag="v")
                for j in range(4):
                    s = g4 * 4 + j
                    nc.tensor.matmul(out=pebP[:, j, :], lhsT=pLT[:, s, :],
                                     rhs=wp2_sb[:], start=True, stop=True)
                nc.vector.tensor_tensor(
                    out=m2[:, g4 * 4:(g4 + 1) * 4, :]
                        .rearrange("p s (g o) -> p s g o", o=CG),
                    in0=pebP[:].rearrange("p s (g o) -> p s g o", o=CG),
                    in1=u[:, g4 * 4:(g4 + 1) * 4, :]
                        .rearrange("p s (g o) -> p s g o", o=1)
                        .broadcast_to([P, 4, G, CG]),
                    op=ALU.mult)
            nc.vector.tensor_tensor(out=macc[:], in0=macc[:], in1=m2[:],
                                    op=ALU.add)

            for hw_ in (8, 4, 2, 1):
                nc.vector.tensor_tensor(out=macc[:, 0:hw_, :],
                                        in0=macc[:, 0:hw_, :],
                                        in1=macc[:, hw_:2 * hw_, :], op=ALU.add)
            fo = sb_sm.tile([P, C], F32, tag="fo")
            nc.vector.tensor_tensor(
                out=fo[:].rearrange("p (g o) -> p g o", o=CG),
                in0=macc[:, 0, :].rearrange("p (g o) -> p g o", o=CG),
                in1=rq[:].rearrange("p (g o) -> p g o", o=1).broadcast_to([P, G, CG]),
                op=ALU.mult)
            nc.sync.dma_start(out=out[r0:r0 + P, :], in_=fo[:])

    nc.finalize()
    return nc


def _center(W):
    """Remove the mean over the output axis (last)."""
    W = np.asarray(W, np.float64)
    return (W - W.mean(axis=-1, keepdims=True)).astype(np.float32)


def _prep_host(q, k, v, xyz, reference_index,
               Wq, bq, gq, betaq, Wk, bk, gk, betak, Wv, bv,
               Wp1, bp1, gp, betap, Wp2, bp2, Ww1, bw1, gw, betaw, Ww2, bw2,
               n_cores):
    for name, arr, val in [
        ("bq", bq, 0), ("gq", gq, 1), ("betaq", betaq, 0),
        ("bk", bk, 0), ("gk", gk, 1), ("betak", betak, 0),
        ("bv", bv, 0), ("bp1", bp1, 0), ("gp", gp, 1), ("betap", betap, 0),
        ("bp2", bp2, 0), ("bw1", bw1, 0), ("gw", gw, 1), ("betaw", betaw, 0),
        ("bw2", bw2, 0),
    ]:
        if not np.allclose(np.asarray(arr), val, atol=1e-6):
            raise NotImplementedError(f"non-trivial {name} not supported")

    N = q.shape[0]
    NR = ((N // n_cores) + 511) // 512 * 512
    NT = (N + 1023) // 1024 * 1024

    def padT(a, rows, dtype=np.float16):
        out = np.zeros((rows, a.shape[1]), dtype=dtype)
        out[:a.shape[0]] = np.asarray(a)
        return out

    kT = np.ascontiguousarray(padT(k, NT).T)      # [C, NT] f16
    vT = np.ascontiguousarray(padT(v, NT).T)
    xyzT = np.ascontiguousarray(padT(xyz, NT).T)  # [3, NT]

    Wq32 = _center(Wq)
    Wk32 = _center(Wk)
    Ww1c = _center(Ww1)
    Wp1c = _center(Wp1)                       # [3, C]
    Wp1c16 = Wp1c.astype(np.float16)
    M3 = (Wp1c16.astype(np.float32) @ Wp1c16.astype(np.float32).T)  # [3,3]
    MqB = np.kron(np.eye(S, dtype=np.float32), M3)                  # [48,48]
    Wp1blk = np.zeros((3 * S, S * C), np.float32)
    for s in range(S):
        Wp1blk[3 * s:3 * s + 3, s * C:(s + 1) * C] = Wp1c
    weights = {
        "Wqc": Wq32.astype(np.float16),
        "Wkc": Wk32.astype(np.float16),
        "Wv": np.asarray(Wv, np.float32).astype(np.float16),
        "Ww1c": Ww1c.astype(np.float16),
        "Wp1blk": Wp1blk.astype(np.float16),
        "MqB": MqB.astype(np.float16),
        "Wp2": np.asarray(Wp2, np.float32).astype(np.float16),
        "Wp2w1": (np.asarray(Wp2, np.float32) @ Ww1c).astype(np.float16),
        "Ww2B": np.kron(np.eye(8, dtype=np.float32),
                        np.asarray(Ww2, np.float32)).astype(np.float16),
    }

    per_core = N // n_cores
    assert per_core * n_cores == N
    ref = np.asarray(reference_index, np.int64)
    in_maps = []
    for i in range(n_cores):
        lo, hi = i * per_core, (i + 1) * per_core
        rsl = ref[lo:hi]                       # [per_core, S]
        nt_tiles = NR // P
        # idx16[t*128+p-like rows, 128]: per 128-row tile, int16 half-indices
        # ordered so gather pair j = s*128+p -> idxs[(j%16) within 16-part
        # block replicated 8x, j//16].
        idx16 = np.zeros((NR, P), np.int16)
        par = np.zeros((NR, S), np.float16)
        half = np.zeros((NR, S), np.int16)
        half[:per_core] = (rsl >> 1).astype(np.int16)
        par[:per_core] = (rsl & 1).astype(np.float16)
        for t in range(nt_tiles):
            blk = half[t * P:(t + 1) * P]          # [128, S]
            lin = blk.T.reshape(-1)                # j = s*128+p
            i16 = lin.reshape(P, 16).T             # [16, 128]
            idx16[t * P:(t + 1) * P] = np.tile(i16, (8, 1))
        tlo, thi = i * (NT // 8), (i + 1) * (NT // 8)
        m = {
            "kTs": np.ascontiguousarray(kT[:, tlo:thi]),
            "vTs": np.ascontiguousarray(vT[:, tlo:thi]),
            "xyzTs": np.ascontiguousarray(xyzT[:, tlo:thi]),
            "qT": np.ascontiguousarray(padT(q[lo:hi], NR).T),
            "xyzsT": np.ascontiguousarray(padT(xyz[lo:hi], NR).T),
            "idx16": idx16,
            "par": par,
        }
        m.update(weights)
        in_maps.append(m)
    return in_maps, NR, NT, per_core


_CACHE = {}


def kernel(**inputs):
    n_cores = 8
    in_maps, NR, NT, per_core = _prep_host(n_cores=n_cores, **inputs)
    key = (NR, NT)
    if key not in _CACHE:
        _CACHE[key] = _build(NR, NT)
    nc = _CACHE[key]
    res = run_bass_kernel_spmd(nc, in_maps, list(range(n_cores)))
    outs = [res.results[i]["out"][:per_core] for i in range(n_cores)]
    return np.ascontiguousarray(np.concatenate(outs, axis=0), dtype=np.float32)
